# revision 7
# baseline (speedup 1.0000x reference)
"""Trainium2 Bass kernel v3: Conv1d(256,256,k=3) -> ReLU -> Linear(256,4) -> CRF Viterbi.

8 cores, data-parallel over batch (8 sequences/core). 81.9us vs v2's 105.1us
(TimelineSim; same cost model as the harness fallback).

Emissions (PE + Act):
  - Conv: fp16 1-term matmuls, (chunk,u)-major psum layout via 2-free-dim
    moving APs, split into u-slabs (u0-31 / u32-47 / u48-63) so most of the
    Viterbi up-sweep can run under the conv.
  - Reversed linear: stationary = relu tile slice [128h, 128t], moving = lt
    [128h, 4]; out psum [128 t-local, 4]. PE cost ~0.2us vs 6.8us for the
    stationary=lt orientation (matmul cost ~ out free size; Ldweights free).
  - Warm-up matmuls on a memset scratch tile (no DMA dependency) ramp the
    PE p-state from ~1.3us; startup-critical input DMAs are split
    (w ho-halves, xt0 hi-halves) so conv starts ~4.4us.

em staging: Act copies psum->SBUF, per-(seq,slab) DMA to a p-flat DRAM
layout [s, q, p_slice, j], then one gather per slab into scan_em
[p = b*16+c, u, j]. The b-major partition map makes both bounce sides
collapse to <=3 AP dims (hardware DMA limit) with 512B runs. em writes go
out on SP; the last one on Act's HWDGE queue to dodge serialization.

Viterbi decode (DVE only -- Pool/gpsimd cannot execute vector ops on HW):
  - M_t[m,j] = trans'[m,j] + em_t[j]; chunk-0 u=0 fixed to (maxplus identity
    + lin_b + em_0) via a host mask add, so G_0 absorbs em_0 and the chain
    init is a replicated constant.
  - Blelloch up-sweep per 64-position chunk; slabs u<48 hidden under conv;
    post-conv remainder is depth-4 plus G = P (x) Uf4[n3] with P = u0..47
    product precomputed under phase 1.
  - G transpose trip: one contiguous write [p, fb, e] + one 16x-duplicating
    gather (0-stride DRAM dim) gives every partition its seq's 16 chunk
    products; fused fwd/bwd mid chains then run full-width, and
    host-precomputed masks select each partition's alpha/beta seed locally
    (no seeds DRAM round trip).
  - Interleaved alpha/beta down-sweeps; tags = argmax_j(alpha+beta) with an
    is_lt/iota trick (ties -> smallest j, matching reference backtracking).
  - pi = b*16+c makes the output DMA fully contiguous.
"""

import numpy as np

import concourse.bass as bass
import concourse.tile as tile
from concourse import mybir
from concourse import bass_utils

B, T, H, K = 64, 1024, 256, 4
NCORES = 8
BPC = B // NCORES
NCH = 16
CL = 64
NEG = -1.0e30
BIG = 1024.0
F32 = mybir.dt.float32
F16 = mybir.dt.float16
I32 = mybir.dt.int32
DUMMY_MMS = 8

# slabs: (u0, nu, q0, nq, cpq)  cpq = chunks per 128-col slice
SLABS = [(0, 32, 0, 4, 4), (32, 16, 4, 2, 8), (48, 16, 6, 2, 8)]


def _ap(t, off, pairs):
    if hasattr(t, "tensor"):
        return bass.AP(tensor=t.tensor, offset=t.offset + off, ap=pairs)
    return bass.AP(tensor=t, offset=off, ap=pairs)


def _apf(t, off, pairs, nparts=None, p0=0):
    p = list(t.ap[0]) if nparts is None else [t.ap[0][0], nparts]
    return bass.AP(tensor=t.tensor, offset=t.offset + p0 * t.ap[0][0] + off,
                   ap=[p] + pairs)


def _split_multi_waits(nc):
    """Walrus allows one semaphore wait per instruction; split extras onto
    same-engine NoOps inserted just before."""
    ctr = 0
    for fn in nc.m.functions:
        for blk in fn.blocks:
            insts = list(blk.instructions)
            new = []
            changed = False
            for inst in insts:
                si = inst.sync_info
                if si is not None and len(si.on_wait) > 1:
                    waits = list(si.on_wait)
                    for w in waits[:-1]:
                        nop = mybir.InstNoOp(name=f"I-ws-{ctr}", ins=[], outs=[])
                        ctr += 1
                        nop.engine = inst.engine
                        nop.sync_info = mybir.SyncInfo(on_wait=[w], on_update=[])
                        new.append(nop)
                    inst.sync_info = mybir.SyncInfo(
                        on_wait=[waits[-1]], on_update=list(si.on_update))
                    changed = True
                new.append(inst)
            if changed:
                blk.instructions = new
    return ctr


def build_program(split_waits=True):
    nc = bass.Bass("TRN2", debug=False, num_devices=NCORES)

    xp = nc.dram_tensor("xp", [BPC, 2, 128, T + 2], F16, kind="ExternalInput")
    wcv = nc.dram_tensor("wcv", [12, 128, 128], F16, kind="ExternalInput")
    lt = nc.dram_tensor("lt", [2, 128, K], F16, kind="ExternalInput")
    cb = nc.dram_tensor("cb", [2, 128], F32, kind="ExternalInput")
    trans_r = nc.dram_tensor("trans_r", [128, 16], F32, kind="ExternalInput")
    e0fix_r = nc.dram_tensor("e0fix_r", [128, 16], F32, kind="ExternalInput")
    iota_r = nc.dram_tensor("iota_r", [128, K], F32, kind="ExternalInput")
    emask_r = nc.dram_tensor("emask_r", [128, 16], F32, kind="ExternalInput")
    wmask_r = nc.dram_tensor("wmask_r", [128, 16], F32, kind="ExternalInput")
    start_r = nc.dram_tensor("start_r", [128, K], F32, kind="ExternalInput")
    end_r = nc.dram_tensor("end_r", [128, K], F32, kind="ExternalInput")
    out_tags = nc.dram_tensor("out_tags", [BPC, T], I32, kind="ExternalOutput")

    from contextlib import ExitStack
    with tile.TileContext(nc) as tc, ExitStack() as ctx:
        consts = ctx.enter_context(tc.tile_pool(name="consts", bufs=1))
        xpool = ctx.enter_context(tc.tile_pool(name="xpool", bufs=8))
        convp = ctx.enter_context(tc.tile_pool(name="convp", bufs=3, space="PSUM"))
        emp = ctx.enter_context(tc.tile_pool(name="emp", bufs=2, space="PSUM"))
        relup = ctx.enter_context(tc.tile_pool(name="relup", bufs=6))
        empool = ctx.enter_context(tc.tile_pool(name="empool", bufs=4))
        spool = ctx.enter_context(tc.tile_pool(name="spool", bufs=1))
        tpool = ctx.enter_context(tc.tile_pool(name="tpool", bufs=4))
        mpool = ctx.enter_context(tc.tile_pool(name="mpool", bufs=3))
        dpool = ctx.enter_context(tc.tile_pool(name="dpool", bufs=1, space="DRAM"))

        # ---- leading DMAs ----
        # warm-up scratch: initialized by a cheap Pool memset (no DMA), so
        # the PE p-state ramp starts almost immediately
        wd_sb = consts.tile([128, 128], F16, tag="wd")
        nc.gpsimd.memset(wd_sb[:, :], 0.0)
        x_tiles = []
        xt0 = xpool.tile([128, 2, T + 2], F16, tag="xt", name="xt0")
        w_sb = consts.tile([128, 12, 128], F16, tag="w")
        # startup-critical splits: ho0 conv weights, then xt0's hi0 half
        nc.sync.dma_start(out=w_sb[:, 0:6, :],
                          in_=_ap(wcv, 0, [[128, 128], [16384, 6], [1, 128]]))
        nc.sync.dma_start(out=xt0[:, 0, :],
                          in_=_ap(xp, 0, [[T + 2, 128], [1, T + 2]]))
        nc.sync.dma_start(out=xt0[:, 1, :],
                          in_=_ap(xp, 128 * (T + 2), [[T + 2, 128], [1, T + 2]]))
        nc.sync.dma_start(out=w_sb[:, 6:12, :],
                          in_=_ap(wcv, 6 * 128 * 128, [[128, 128], [16384, 6], [1, 128]]))
        x_tiles.append(xt0)
        lt_sb = consts.tile([128, 2, K], F16, tag="lt")
        nc.sync.dma_start(out=lt_sb[:, :, :], in_=lt.ap().rearrange("h p j -> p h j"))
        cb_sb = consts.tile([128, 2], F32, tag="cb")
        nc.sync.dma_start(out=cb_sb[:, :], in_=cb.ap().rearrange("h p -> p h"))
        for s in range(1, BPC):
            xt = xpool.tile([128, 2, T + 2], F16, tag="xt", name=f"xt{s}")
            nc.sync.dma_start(out=xt[:, :, :],
                              in_=xp.ap()[s].rearrange("g p t -> p g t"))
            x_tiles.append(xt)
        trans_sb = consts.tile([128, 16], F32, tag="tr")
        nc.sync.dma_start(out=trans_sb[:, :], in_=trans_r.ap())
        e0fix_sb = consts.tile([128, 16], F32, tag="e0f")
        nc.sync.dma_start(out=e0fix_sb[:, :], in_=e0fix_r.ap())
        iota_sb = consts.tile([128, K], F32, tag="io")
        nc.sync.dma_start(out=iota_sb[:, :], in_=iota_r.ap())
        emask_sb = consts.tile([128, 16], F32, tag="em")
        nc.sync.dma_start(out=emask_sb[:, :], in_=emask_r.ap())
        wmask_sb = consts.tile([128, 16], F32, tag="wm")
        nc.sync.dma_start(out=wmask_sb[:, :], in_=wmask_r.ap())
        start_sb = consts.tile([128, K], F32, tag="sst")
        nc.sync.dma_start(out=start_sb[:, :], in_=start_r.ap())
        end_sb = consts.tile([128, K], F32, tag="send")
        nc.sync.dma_start(out=end_sb[:, :], in_=end_r.ap())

        dram_em = dpool.tile([BPC, 8, 128, K], F32, tag="dem")
        dram_g = dpool.tile([128, 2, 16], F32, tag="dg")

        # ---- persistent scan tiles ----
        scan_em = spool.tile([128, CL, K], F32, tag="sem")  # [p=(b,c), u, j]
        sp_ = scan_em.ap[0][0]
        M = spool.tile([128, CL, K, K], F32, tag="M")
        Uf = [M]
        for lvl in range(1, 7):
            n = CL >> lvl
            Uf.append(spool.tile([128, n * 16], F32, tag=f"Uf{lvl}",
                                 name=f"Uf{lvl}"))
        G = Uf[6]
        Gp = spool.tile([128, 16], F32, tag="Gp")
        Gp2 = spool.tile([128, 16], F32, tag="Gp2")

        # ---- PE warm-up ----
        warm = convp.tile([128, 128], F32, tag="warm")
        for _ in range(DUMMY_MMS):
            nc.tensor.matmul(warm[:, :], wd_sb[:, :], wd_sb[:, :],
                             start=True, stop=True)

        def product(dst, d_off, srcA, a_off, srcB, b_off, use_pool=False):
            """dst[d_off] = srcA[a_off] (x) srcB[b_off], single 4x4 node."""
            tmp = tpool.tile([128, 64], F32, tag="tmp1")
            for i in range(4):
                eng = nc.gpsimd if (use_pool and i >= 2) else nc.vector
                eng.tensor_tensor(
                    out=_apf(tmp, i * 16, [[64, 1], [4, 4], [1, 4]]),
                    in0=_apf(srcA, a_off + i * 4, [[32, 1], [0, 4], [1, 4]]),
                    in1=_apf(srcB, b_off, [[32, 1], [1, 4], [4, 4]]),
                    op=mybir.AluOpType.add,
                )
            nc.vector.tensor_reduce(
                out=_apf(dst, d_off, [[16, 1], [4, 4], [1, 4]]),
                in_=_apf(tmp, 0, [[64, 1], [16, 4], [4, 4], [1, 4]]),
                axis=mybir.AxisListType.X, op=mybir.AluOpType.max,
            )

        def tree_level(lvl, v0, n, use_pool=False):
            prev = Uf[lvl - 1]
            tmp = tpool.tile([128, n * 64], F32, tag="tmp")
            for i in range(4):
                eng = nc.gpsimd if (use_pool and i >= 2) else nc.vector
                eng.tensor_tensor(
                    out=_apf(tmp, i * 16, [[64, n], [4, 4], [1, 4]]),
                    in0=_apf(prev, v0 * 32 + i * 4, [[32, n], [0, 4], [1, 4]]),
                    in1=_apf(prev, v0 * 32 + 16, [[32, n], [1, 4], [4, 4]]),
                    op=mybir.AluOpType.add,
                )
            nc.vector.tensor_reduce(
                out=_apf(Uf[lvl], v0 * 16, [[16, n], [4, 4], [1, 4]]),
                in_=_apf(tmp, 0, [[64, n], [16, 4], [4, 4], [1, 4]]),
                axis=mybir.AxisListType.X, op=mybir.AluOpType.max,
            )

        def emit_conv(si, xt, u0, nu):
            """conv for (seq si, slab): returns relu tiles per ho."""
            rus = {}
            for ho in range(2):
                cp = convp.tile([128, 16 * nu], F32, tag="cp")
                for i in range(6):
                    hi, k = i // 3, i % 3
                    g = ho * 6 + k * 2 + hi
                    nc.tensor.matmul(
                        cp[:, :],
                        w_sb[:, g, :],
                        _ap(xt, hi * (T + 2) + k + u0, [[xt.ap[0][0], 128],
                                                        [64, 16], [1, nu]]),
                        start=(i == 0), stop=(i == 5),
                    )
                ru = relup.tile([128, 16 * nu], F16, tag="ru")
                nc.scalar.activation(
                    out=ru[:, :], in_=cp[:, :],
                    func=mybir.ActivationFunctionType.Relu,
                    bias=cb_sb[:, ho:ho + 1], scale=1.0,
                )
                rus[ho] = ru
            return rus

        def emit_linear(si, rus, q0, nq, via_act=False):
            """linear + em copy + em write for (seq si, slab)."""
            eps = emp.tile([128, nq, K], F32, tag="eps")
            for q in range(nq):
                for ho in range(2):
                    nc.tensor.matmul(
                        eps[:, q, :],
                        rus[ho][:, q * 128:(q + 1) * 128],
                        lt_sb[:, ho, :],
                        start=(ho == 0), stop=(ho == 1),
                    )
            em_sb = empool.tile([128, nq, K], F32, tag="esb")
            nc.scalar.copy(out=em_sb[:, :, :], in_=eps[:, :, :])
            # dram_em [s, qg, p_slice, j]: addr = s*4096 + qg*512 + p*4 + j
            eng = nc.scalar if via_act else nc.sync
            eng.dma_start(
                out=_ap(dram_em, si * 4096 + q0 * 512,
                        [[K, 128], [512, nq], [1, K]]),
                in_=em_sb[:, :, :],
            )

        def emit_gather(slab):
            u0, nu, q0, nq, cpq = SLABS[slab]
            # out: scan_em[:, u0:u0+nu, :] ; partition map pi = b*16 + c
            # in: iter (b, q, cl, u, j) -> addr = b*4096 + (q0+q)*512
            #     + (cl*nu+u)*4 + j ; (q,cl,u,j) collapses to one contiguous
            #     run of nq*512 els per b.
            nc.sync.dma_start(
                out=_ap(scan_em, u0 * K, [[sp_, 128], [1, nu * K]]),
                in_=_ap(dram_em, q0 * 512, [[4096, BPC], [1, nq * 512]]),
            )

        def emit_tree(slab):
            u0, nu, q0, nq, cpq = SLABS[slab]
            # M build for this u range
            nc.vector.tensor_tensor(
                out=_apf(M, u0 * 16, [[16, nu], [4, 4], [1, 4]]),
                in0=_apf(scan_em, u0 * K, [[K, nu], [0, 4], [1, 4]]),
                in1=_apf(trans_sb, 0, [[0, nu], [4, 4], [1, 4]]),
                op=mybir.AluOpType.add,
            )
            if slab == 0:
                # chunk-0 u=0: M_0 = identity + lin_b + em_0. e0fix is
                # (e_mat - crf_trans) on chunk-0 partitions (p % 16 == 0),
                # 0 elsewhere, so a full-width add fixes only chunk 0.
                nc.vector.tensor_tensor(
                    out=_apf(M, 0, [[4, 4], [1, 4]]),
                    in0=_apf(M, 0, [[4, 4], [1, 4]]),
                    in1=_apf(e0fix_sb, 0, [[4, 4], [1, 4]]),
                    op=mybir.AluOpType.add,
                )
                tree_level(1, 0, 16)
                tree_level(2, 0, 8)
                tree_level(3, 0, 4)
                tree_level(4, 0, 2)
                tree_level(5, 0, 1)
            elif slab == 1:
                tree_level(1, 16, 8)
                tree_level(2, 8, 4)
                tree_level(3, 4, 2)
                tree_level(4, 2, 1)
                # P = u0..47 product = Uf5[n0] (x) Uf4[n2]  (hidden under phase 1)
                product(Gp, 0, Uf[5], 0, Uf[4], 2 * 16)
            else:
                tree_level(1, 24, 8)
                tree_level(2, 12, 4)
                tree_level(3, 6, 2)
                tree_level(4, 3, 1)
                # G shortcut: depth 5 on the critical path
                product(G, 0, Gp, 0, Uf[4], 3 * 16)

        # ---- phase 1: slab-outer, seq-inner; linear deferred one unit ----
        pending = None  # (si, rus, slab)
        for slab in range(3):
            u0, nu, q0, nq, cpq = SLABS[slab]
            for si in range(BPC):
                rus = emit_conv(si, x_tiles[si], u0, nu)
                if pending is not None:
                    psi, prus, pslab = pending
                    pu0, pnu, pq0, pnq, pcpq = SLABS[pslab]
                    emit_linear(psi, prus, pq0, pnq)
                    if psi == BPC - 1:
                        emit_gather(pslab)
                        emit_tree(pslab)
                pending = (si, rus, slab)
        psi, prus, pslab = pending
        emit_linear(psi, prus, SLABS[pslab][2], SLABS[pslab][3], via_act=True)
        emit_gather(2)
        emit_tree(2)

        # ---- G transpose trip ----
        GGT = spool.tile([128, 2, 16], F32, tag="GGT")
        nc.vector.tensor_copy(out=GGT[:, 0, :], in_=G[:, :])
        nc.vector.tensor_copy(out=GGT[:, 1, :], in_=_apf(G, 0, [[1, 4], [4, 4]]))
        # off-path node: Uf5[n1] (beta d=0 needs it)
        tree_level(5, 1, 1)
        # write: dram_g[p, fb, e] contiguous (p = b*16 + c)
        nc.sync.dma_start(
            out=_ap(dram_g, 0, [[32, 128], [1, 32]]),
            in_=GGT[:, :, :],
        )
        # dup gather: gm[p', c'', fb, e] = dram_g[b(p')*16 + c'', fb, e]
        # iter (b, c-dup, c'', fb, e): [[512, 8], [0, 16], [1, 512]]
        gm = spool.tile([128, NCH, 2, 16], F32, tag="gm")
        nc.sync.dma_start(
            out=gm[:, :, :, :],
            in_=_ap(dram_g, 0, [[512, 8], [0, 16], [1, 512]]),
        )

        # ---- full-width fused mid chains ----
        stfb = spool.tile([128, NCH, 2, K], F32, tag="stfb")
        nc.vector.tensor_copy(out=stfb[:, 0, 0, :], in_=start_sb[:, :])
        nc.vector.tensor_copy(out=stfb[:, 0, 1, :], in_=end_sb[:, :])
        for i in range(1, NCH):
            tmpm = mpool.tile([128, 2, K, K], F32, tag="mtm")
            # gm free layout (c', fb, e): fwd -> gm[i-1, 0, :] at (i-1)*32;
            # bwd -> gm[16-i, 1, :] at (16-i)*32 + 16
            fbs = (16 - i) * 32 + 16 - (i - 1) * 32
            nc.vector.tensor_tensor(
                out=tmpm[:, :, :, :],
                in0=_apf(stfb, (i - 1) * 2 * K, [[K, 2], [0, 4], [1, 4]]),
                in1=_apf(gm, (i - 1) * 32, [[fbs, 2], [1, 4], [4, 4]]),
                op=mybir.AluOpType.add,
            )
            nc.vector.tensor_reduce(out=stfb[:, i, :, :], in_=tmpm[:, :, :, :],
                                    axis=mybir.AxisListType.X,
                                    op=mybir.AluOpType.max)

        # ---- local seed select (host masks; no DRAM trip) ----
        sv = spool.tile([128, CL + 1, K], F32, tag="sv")
        wv = spool.tile([128, CL, K], F32, tag="wv")
        selt = mpool.tile([128, K, NCH], F32, tag="selt")
        # alpha seed: sv[:,0,j] = sum_c stfb[:,c,0,j] * emask[:,c]
        nc.vector.tensor_tensor(
            out=selt[:, :, :],
            in0=_apf(stfb, 0, [[1, 4], [8, 16]]),
            in1=_apf(emask_sb, 0, [[0, 4], [1, 16]]),
            op=mybir.AluOpType.mult,
        )
        nc.vector.tensor_reduce(out=sv[:, 0, :], in_=selt[:, :, :],
                                axis=mybir.AxisListType.X, op=mybir.AluOpType.add)
        # beta seed: wv[:,CL-1,j] = sum_c stfb[:,c,1,j] * wmask[:,c]
        selt2 = mpool.tile([128, K, NCH], F32, tag="selt2")
        nc.vector.tensor_tensor(
            out=selt2[:, :, :],
            in0=_apf(stfb, K, [[1, 4], [8, 16]]),
            in1=_apf(wmask_sb, 0, [[0, 4], [1, 16]]),
            op=mybir.AluOpType.mult,
        )
        nc.vector.tensor_reduce(out=wv[:, CL - 1, :], in_=selt2[:, :, :],
                                axis=mybir.AxisListType.X, op=mybir.AluOpType.add)

        # ---- down-sweeps, alpha/beta interleaved ----
        t64 = tpool.tile([128, 4, 4], F32, tag="t64")
        nc.vector.tensor_tensor(
            out=t64[:, :, :],
            in0=_apf(sv, 0, [[0, 4], [1, 4]]),
            in1=_apf(G, 0, [[1, 4], [4, 4]]),
            op=mybir.AluOpType.add,
        )
        nc.vector.tensor_reduce(out=sv[:, CL, :], in_=t64[:, :, :],
                                axis=mybir.AxisListType.X, op=mybir.AluOpType.max)
        for d in range(6):
            s_ = CL >> d
            n = 1 << d
            tmpa = tpool.tile([128, n, 4, 4], F32, tag="tmpa")
            nc.vector.tensor_tensor(
                out=tmpa[:, :, :, :],
                in0=_apf(sv, 0, [[s_ * 4, n], [0, 4], [1, 4]]),
                in1=_apf(Uf[5 - d], 0, [[32, n], [1, 4], [4, 4]]),
                op=mybir.AluOpType.add,
            )
            nc.vector.tensor_reduce(
                out=_apf(sv, (s_ // 2) * 4, [[s_ * 4, n], [1, 4]]),
                in_=tmpa[:, :, :, :],
                axis=mybir.AxisListType.X, op=mybir.AluOpType.max,
            )
            tmpb = tpool.tile([128, n, 4, 4], F32, tag="tmpb")
            nc.vector.tensor_tensor(
                out=tmpb[:, :, :, :],
                in0=_apf(wv, (s_ - 1) * 4, [[s_ * 4, n], [0, 4], [1, 4]]),
                in1=_apf(Uf[5 - d], 16, [[32, n], [4, 4], [1, 4]]),
                op=mybir.AluOpType.add,
            )
            nc.vector.tensor_reduce(
                out=_apf(wv, (s_ // 2 - 1) * 4, [[s_ * 4, n], [1, 4]]),
                in_=tmpb[:, :, :, :],
                axis=mybir.AxisListType.X, op=mybir.AluOpType.max,
            )

        # ---- tags: argmax_j(alpha + beta), ties -> smallest j ----
        t4 = spool.tile([128, CL, K], F32, tag="t4")
        nc.vector.tensor_tensor(out=t4[:, :, :],
                                in0=_apf(sv, K, [[1, CL * K]]),
                                in1=_apf(wv, 0, [[1, CL * K]]),
                                op=mybir.AluOpType.add)
        rm = spool.tile([128, CL], F32, tag="rm")
        nc.vector.tensor_reduce(out=rm[:, :], in_=t4[:, :, :],
                                axis=mybir.AxisListType.X, op=mybir.AluOpType.max)
        eq = spool.tile([128, CL, K], F32, tag="eq")
        nc.vector.tensor_tensor(out=eq[:, :, :], in0=t4[:, :, :],
                                in1=_apf(rm, 0, [[1, CL], [0, K]]),
                                op=mybir.AluOpType.is_lt)
        t5 = spool.tile([128, CL, K], F32, tag="t5")
        nc.vector.scalar_tensor_tensor(
            out=t5[:, :, :], in0=eq[:, :, :], scalar=BIG,
            in1=_apf(iota_sb, 0, [[0, CL], [1, K]]),
            op0=mybir.AluOpType.mult, op1=mybir.AluOpType.add)
        amn = spool.tile([128, CL], F32, tag="amn")
        nc.vector.tensor_reduce(out=amn[:, :], in_=t5[:, :, :],
                                axis=mybir.AxisListType.X, op=mybir.AluOpType.min)
        tagi = spool.tile([128, CL], I32, tag="tagi")
        nc.vector.tensor_copy(out=tagi[:, :], in_=amn[:, :])
        # pi = b*16 + c: out addr = p*64 + u (fully contiguous)
        nc.sync.dma_start(
            out=_ap(out_tags, 0, [[CL, 128], [1, CL]]),
            in_=tagi[:, :],
        )

    if split_waits:
        _split_multi_waits(nc)
    return nc


def prep_core_inputs(core, sentence_features, conv_w, conv_b, lin_w, lin_b,
                     crf_start, crf_end, crf_trans):
    sf = np.asarray(sentence_features, np.float32)
    conv_w = np.asarray(conv_w, np.float32)
    conv_b = np.asarray(conv_b, np.float32)
    lin_w = np.asarray(lin_w, np.float32)
    lin_b = np.asarray(lin_b, np.float32)
    crf_start = np.asarray(crf_start, np.float32)
    crf_end = np.asarray(crf_end, np.float32)
    crf_trans = np.asarray(crf_trans, np.float32)

    xsh = sf[core * BPC:(core + 1) * BPC]  # [8, T, H]
    xpad = np.zeros((BPC, H, T + 2), np.float32)
    xpad[:, :, 1:T + 1] = xsh.transpose(0, 2, 1)
    xp = np.ascontiguousarray(xpad.reshape(BPC, 2, 128, T + 2)).astype(np.float16)

    wt = conv_w.transpose(1, 0, 2)  # [hin, hout, k]
    wcv = np.empty((12, 128, 128), np.float32)
    for ho in range(2):
        for k in range(3):
            for hi in range(2):
                g = ho * 6 + k * 2 + hi
                wcv[g] = wt[hi * 128:(hi + 1) * 128, ho * 128:(ho + 1) * 128, k]

    transp = crf_trans + lin_b[None, :]
    e_mat = np.full((4, 4), NEG, np.float32)
    np.fill_diagonal(e_mat, 0.0)
    # partition map pi = b*16 + c: c = p % 16
    cvec = np.arange(128) % 16
    emask = np.zeros((128, 16), np.float32)
    emask[np.arange(128), cvec] = 1.0
    wmask = np.zeros((128, 16), np.float32)
    wmask[np.arange(128), 15 - cvec] = 1.0
    e0fix = np.zeros((128, 16), np.float32)
    e0fix[cvec == 0] = (e_mat - crf_trans).reshape(16)

    return {
        "xp": xp,
        "wcv": wcv.astype(np.float16),
        "lt": np.ascontiguousarray(lin_w.T.reshape(2, 128, K)).astype(np.float16),
        "cb": conv_b.reshape(2, 128).copy(),
        "trans_r": np.tile(transp.reshape(1, 16), (128, 1)).copy(),
        "e0fix_r": e0fix,
        "iota_r": np.tile(np.arange(4, dtype=np.float32), (128, 1)).copy(),
        "emask_r": emask,
        "wmask_r": wmask,
        "start_r": np.tile(crf_start[None, :], (128, 1)).astype(np.float32),
        "end_r": np.tile(crf_end[None, :], (128, 1)).astype(np.float32),
    }


_NC_CACHE = None


def kernel(sentence_features, conv_w, conv_b, lin_w, lin_b, crf_start,
           crf_end, crf_trans):
    global _NC_CACHE
    if _NC_CACHE is None:
        _NC_CACHE = build_program()
    nc = _NC_CACHE
    in_maps = [
        prep_core_inputs(c, sentence_features, conv_w, conv_b, lin_w, lin_b,
                         crf_start, crf_end, crf_trans)
        for c in range(NCORES)
    ]
    res = bass_utils.run_bass_kernel_spmd(nc, in_maps, core_ids=list(range(NCORES)))
    kernel.last_results = res
    out = np.concatenate([res.results[c]["out_tags"] for c in range(NCORES)], axis=0)
    return out.astype(np.int32)


# revision 9
# speedup vs baseline: 1.0359x; 1.0359x over previous
"""Trainium2 Bass kernel v3: Conv1d(256,256,k=3) -> ReLU -> Linear(256,4) -> CRF Viterbi.

8 cores, data-parallel over batch (8 sequences/core). 81.9us vs v2's 105.1us
(TimelineSim; same cost model as the harness fallback).

Emissions (PE + Act):
  - Conv: fp16 1-term matmuls, (chunk,u)-major psum layout via 2-free-dim
    moving APs, split into u-slabs (u0-31 / u32-47 / u48-63) so most of the
    Viterbi up-sweep can run under the conv.
  - Reversed linear: stationary = relu tile slice [128h, 128t], moving = lt
    [128h, 4]; out psum [128 t-local, 4]. PE cost ~0.2us vs 6.8us for the
    stationary=lt orientation (matmul cost ~ out free size; Ldweights free).
  - Warm-up matmuls on a memset scratch tile (no DMA dependency) ramp the
    PE p-state from ~1.3us; startup-critical input DMAs are split
    (w ho-halves, xt0 hi-halves) so conv starts ~4.4us.

em staging: Act copies psum->SBUF, per-(seq,slab) DMA to a p-flat DRAM
layout [s, q, p_slice, j], then one gather per slab into scan_em
[p = b*16+c, u, j]. The b-major partition map makes both bounce sides
collapse to <=3 AP dims (hardware DMA limit) with 512B runs. em writes go
out on SP; the last one on Act's HWDGE queue to dodge serialization.

Viterbi decode (DVE only -- Pool/gpsimd cannot execute vector ops on HW):
  - M_t[m,j] = trans'[m,j] + em_t[j]; chunk-0 u=0 fixed to (maxplus identity
    + lin_b + em_0) via a host mask add, so G_0 absorbs em_0 and the chain
    init is a replicated constant.
  - Blelloch up-sweep per 64-position chunk; slabs u<48 hidden under conv;
    post-conv remainder is depth-4 plus G = P (x) Uf4[n3] with P = u0..47
    product precomputed under phase 1.
  - G transpose trip: one contiguous write [p, fb, e] + one 16x-duplicating
    gather (0-stride DRAM dim) gives every partition its seq's 16 chunk
    products; fused fwd/bwd mid chains then run full-width, and
    host-precomputed masks select each partition's alpha/beta seed locally
    (no seeds DRAM round trip).
  - Interleaved alpha/beta down-sweeps; tags = argmax_j(alpha+beta) with an
    is_lt/iota trick (ties -> smallest j, matching reference backtracking).
  - pi = b*16+c makes the output DMA fully contiguous.
"""

import numpy as np

import concourse.bass as bass
import concourse.tile as tile
from concourse import mybir
from concourse import bass_utils

B, T, H, K = 64, 1024, 256, 4
NCORES = 8
BPC = B // NCORES
NCH = 16
CL = 64
NEG = -1.0e30
BIG = 1024.0
F32 = mybir.dt.float32
F16 = mybir.dt.float16
I32 = mybir.dt.int32
DUMMY_MMS = 8

# slabs: (u0, nu, q0, nq, cpq)  cpq = chunks per 128-col slice
SLABS = [(0, 32, 0, 4, 4), (32, 16, 4, 2, 8), (48, 16, 6, 2, 8)]


def _ap(t, off, pairs):
    if hasattr(t, "tensor"):
        return bass.AP(tensor=t.tensor, offset=t.offset + off, ap=pairs)
    return bass.AP(tensor=t, offset=off, ap=pairs)


def _apf(t, off, pairs, nparts=None, p0=0):
    p = list(t.ap[0]) if nparts is None else [t.ap[0][0], nparts]
    return bass.AP(tensor=t.tensor, offset=t.offset + p0 * t.ap[0][0] + off,
                   ap=[p] + pairs)


def _split_multi_waits(nc):
    """Walrus allows one semaphore wait per instruction; split extras onto
    same-engine NoOps inserted just before."""
    ctr = 0
    for fn in nc.m.functions:
        for blk in fn.blocks:
            insts = list(blk.instructions)
            new = []
            changed = False
            for inst in insts:
                si = inst.sync_info
                if si is not None and len(si.on_wait) > 1:
                    waits = list(si.on_wait)
                    for w in waits[:-1]:
                        nop = mybir.InstNoOp(name=f"I-ws-{ctr}", ins=[], outs=[])
                        ctr += 1
                        nop.engine = inst.engine
                        nop.sync_info = mybir.SyncInfo(on_wait=[w], on_update=[])
                        new.append(nop)
                    inst.sync_info = mybir.SyncInfo(
                        on_wait=[waits[-1]], on_update=list(si.on_update))
                    changed = True
                new.append(inst)
            if changed:
                blk.instructions = new
    return ctr


def build_program(split_waits=True):
    nc = bass.Bass("TRN2", debug=False, num_devices=NCORES)

    xp = nc.dram_tensor("xp", [BPC, 2, 128, T + 2], F16, kind="ExternalInput")
    wcv = nc.dram_tensor("wcv", [12, 128, 128], F16, kind="ExternalInput")
    lt = nc.dram_tensor("lt", [2, 128, K], F16, kind="ExternalInput")
    cb = nc.dram_tensor("cb", [2, 128], F32, kind="ExternalInput")
    trans_r = nc.dram_tensor("trans_r", [128, 16], F32, kind="ExternalInput")
    trans2_r = nc.dram_tensor("trans2_r", [128, 64], F32, kind="ExternalInput")
    e0fix_r = nc.dram_tensor("e0fix_r", [128, 16], F32, kind="ExternalInput")
    iota_r = nc.dram_tensor("iota_r", [128, K], F32, kind="ExternalInput")
    emask_r = nc.dram_tensor("emask_r", [128, 16], F32, kind="ExternalInput")
    wmask_r = nc.dram_tensor("wmask_r", [128, 16], F32, kind="ExternalInput")
    start_r = nc.dram_tensor("start_r", [128, K], F32, kind="ExternalInput")
    end_r = nc.dram_tensor("end_r", [128, K], F32, kind="ExternalInput")
    out_tags = nc.dram_tensor("out_tags", [BPC, T], I32, kind="ExternalOutput")

    from contextlib import ExitStack
    with tile.TileContext(nc) as tc, ExitStack() as ctx:
        consts = ctx.enter_context(tc.tile_pool(name="consts", bufs=1))
        xpool = ctx.enter_context(tc.tile_pool(name="xpool", bufs=8))
        convp = ctx.enter_context(tc.tile_pool(name="convp", bufs=3, space="PSUM"))
        emp = ctx.enter_context(tc.tile_pool(name="emp", bufs=1, space="PSUM"))
        em2pool = ctx.enter_context(tc.tile_pool(name="em2pool", bufs=1, space="PSUM"))
        relup = ctx.enter_context(tc.tile_pool(name="relup", bufs=10))
        empool = ctx.enter_context(tc.tile_pool(name="empool", bufs=4))
        spool = ctx.enter_context(tc.tile_pool(name="spool", bufs=1))
        tpool = ctx.enter_context(tc.tile_pool(name="tpool", bufs=4))
        mpool = ctx.enter_context(tc.tile_pool(name="mpool", bufs=3))
        dpool = ctx.enter_context(tc.tile_pool(name="dpool", bufs=1, space="DRAM"))

        # ---- leading DMAs ----
        # warm-up scratch: initialized by a cheap Pool memset (no DMA), so
        # the PE p-state ramp starts almost immediately
        wd_sb = consts.tile([128, 128], F16, tag="wd")
        nc.gpsimd.memset(wd_sb[:, :], 0.0)
        x_tiles = []
        SEQB = 2 * 128 * (T + 2)  # xp seq stride (els)
        xt0 = xpool.tile([128, 2, 2, T + 2], F16, tag="xt", name="xt0")
        w_sb = consts.tile([128, 12, 128], F16, tag="w")
        # startup-critical splits: ho0 conv weights, then seq0's hi0 half
        nc.sync.dma_start(out=w_sb[:, 0:6, :],
                          in_=_ap(wcv, 0, [[128, 128], [16384, 6], [1, 128]]))
        nc.sync.dma_start(out=_apf(xt0, 0, [[1, T + 2]]),
                          in_=_ap(xp, 0, [[T + 2, 128], [1, T + 2]]))
        nc.sync.dma_start(out=_apf(xt0, T + 2, [[1, 3 * (T + 2)]]),
                          in_=_ap(xp, 128 * (T + 2),
                                  [[T + 2, 128], [SEQB // 2, 3], [1, T + 2]]))
        nc.sync.dma_start(out=w_sb[:, 6:12, :],
                          in_=_ap(wcv, 6 * 128 * 128, [[128, 128], [16384, 6], [1, 128]]))
        x_tiles.append(xt0)
        lt_sb = consts.tile([128, 2, K], F16, tag="lt")
        nc.sync.dma_start(out=lt_sb[:, :, :], in_=lt.ap().rearrange("h p j -> p h j"))
        cb_sb = consts.tile([128, 2], F32, tag="cb")
        nc.sync.dma_start(out=cb_sb[:, :], in_=cb.ap().rearrange("h p -> p h"))
        for kk in range(1, 4):
            xt = xpool.tile([128, 2, 2, T + 2], F16, tag="xt", name=f"xt{kk}")
            nc.sync.dma_start(
                out=xt[:, :, :, :],
                in_=_ap(xp, 2 * kk * SEQB, [[T + 2, 128], [SEQB // 2, 4],
                                            [1, T + 2]]))
            x_tiles.append(xt)
        trans_sb = consts.tile([128, 16], F32, tag="tr")
        nc.sync.dma_start(out=trans_sb[:, :], in_=trans_r.ap())
        trans2_sb = consts.tile([128, 4, 16], F32, tag="tr2")
        nc.sync.dma_start(out=trans2_sb[:, :, :], in_=trans2_r.ap())
        e0fix_sb = consts.tile([128, 16], F32, tag="e0f")
        nc.sync.dma_start(out=e0fix_sb[:, :], in_=e0fix_r.ap())
        iota_sb = consts.tile([128, K], F32, tag="io")
        nc.sync.dma_start(out=iota_sb[:, :], in_=iota_r.ap())
        emask_sb = consts.tile([128, 16], F32, tag="em")
        nc.sync.dma_start(out=emask_sb[:, :], in_=emask_r.ap())
        wmask_sb = consts.tile([128, 16], F32, tag="wm")
        nc.sync.dma_start(out=wmask_sb[:, :], in_=wmask_r.ap())
        start_sb = consts.tile([128, K], F32, tag="sst")
        nc.sync.dma_start(out=start_sb[:, :], in_=start_r.ap())
        end_sb = consts.tile([128, K], F32, tag="send")
        nc.sync.dma_start(out=end_sb[:, :], in_=end_r.ap())

        dram_em = dpool.tile([BPC, 8, 128, K], F32, tag="dem")
        dram_g = dpool.tile([128, 2, 16], F32, tag="dg")

        # ---- persistent scan tiles ----
        scan_em = spool.tile([128, CL, K], F32, tag="sem")  # [p=(b,c), u, j]
        em2p = em2pool.tile([128, 16, K], F32, tag="em2p")
        sp_ = scan_em.ap[0][0]
        M = spool.tile([128, CL, K, K], F32, tag="M")
        Uf = [M]
        for lvl in range(1, 7):
            n = CL >> lvl
            Uf.append(spool.tile([128, n * 16], F32, tag=f"Uf{lvl}",
                                 name=f"Uf{lvl}"))
        G = Uf[6]
        Gp = spool.tile([128, 16], F32, tag="Gp")
        GGT = spool.tile([128, 2, 16], F32, tag="GGT")
        Gp2 = spool.tile([128, 16], F32, tag="Gp2")

        # ---- PE warm-up ----
        warm = convp.tile([128, 128], F32, tag="warm")
        for _ in range(DUMMY_MMS):
            nc.tensor.matmul(warm[:, :], wd_sb[:, :], wd_sb[:, :],
                             start=True, stop=True)

        def product(dst, d_off, srcA, a_off, srcB, b_off, use_pool=False):
            """dst[d_off] = srcA[a_off] (x) srcB[b_off], single 4x4 node."""
            tmp = tpool.tile([128, 64], F32, tag="tmp1")
            for i in range(4):
                eng = nc.gpsimd if (use_pool and i >= 2) else nc.vector
                eng.tensor_tensor(
                    out=_apf(tmp, i * 16, [[64, 1], [4, 4], [1, 4]]),
                    in0=_apf(srcA, a_off + i * 4, [[32, 1], [0, 4], [1, 4]]),
                    in1=_apf(srcB, b_off, [[32, 1], [1, 4], [4, 4]]),
                    op=mybir.AluOpType.add,
                )
            nc.vector.tensor_reduce(
                out=_apf(dst, d_off, [[16, 1], [4, 4], [1, 4]]),
                in_=_apf(tmp, 0, [[64, 1], [16, 4], [4, 4], [1, 4]]),
                axis=mybir.AxisListType.X, op=mybir.AluOpType.max,
            )

        def tree_level(lvl, v0, n, use_pool=False):
            prev = Uf[lvl - 1]
            tmp = tpool.tile([128, n * 64], F32, tag="tmp")
            for i in range(4):
                eng = nc.gpsimd if (use_pool and i >= 2) else nc.vector
                eng.tensor_tensor(
                    out=_apf(tmp, i * 16, [[64, n], [4, 4], [1, 4]]),
                    in0=_apf(prev, v0 * 32 + i * 4, [[32, n], [0, 4], [1, 4]]),
                    in1=_apf(prev, v0 * 32 + 16, [[32, n], [1, 4], [4, 4]]),
                    op=mybir.AluOpType.add,
                )
            nc.vector.tensor_reduce(
                out=_apf(Uf[lvl], v0 * 16, [[16, n], [4, 4], [1, 4]]),
                in_=_apf(tmp, 0, [[64, n], [16, 4], [4, 4], [1, 4]]),
                axis=mybir.AxisListType.X, op=mybir.AluOpType.max,
            )

        def emit_conv(si, xt, u0, nu):
            soff = (si % 2) * 2 * (T + 2)
            """conv for (seq si, slab): returns relu tiles per ho."""
            rus = {}
            for ho in range(2):
                cp = convp.tile([128, 16 * nu], F32, tag="cp")
                for i in range(6):
                    hi, k = i // 3, i % 3
                    g = ho * 6 + k * 2 + hi
                    nc.tensor.matmul(
                        cp[:, :],
                        w_sb[:, g, :],
                        _ap(xt, soff + hi * (T + 2) + k + u0,
                            [[xt.ap[0][0], 128], [64, 16], [1, nu]]),
                        start=(i == 0), stop=(i == 5),
                    )
                ru = relup.tile([128, 16 * nu], F16, tag="ru")
                nc.scalar.activation(
                    out=ru[:, :], in_=cp[:, :],
                    func=mybir.ActivationFunctionType.Relu,
                    bias=cb_sb[:, ho:ho + 1], scale=1.0,
                )
                rus[ho] = ru
            return rus

        def emit_conv_pair(k, u0, nu):
            # both seqs of pair k in one matmul set: psum cols (s, c, u)
            xt = x_tiles[k]
            rus = {}
            for ho in range(2):
                cp = convp.tile([128, 32 * nu], F32, tag="cp")
                for i in range(6):
                    hi, kk = i // 3, i % 3
                    g = ho * 6 + kk * 2 + hi
                    nc.tensor.matmul(
                        cp[:, :],
                        w_sb[:, g, :],
                        _ap(xt, hi * (T + 2) + kk + u0,
                            [[xt.ap[0][0], 128], [2 * (T + 2), 2],
                             [64, 16], [1, nu]]),
                        start=(i == 0), stop=(i == 5),
                    )
                ru = relup.tile([128, 32 * nu], F16, tag="ru")
                nc.scalar.activation(
                    out=ru[:, :], in_=cp[:, :],
                    func=mybir.ActivationFunctionType.Relu,
                    bias=cb_sb[:, ho:ho + 1], scale=1.0,
                )
                rus[ho] = ru
            return rus

        def emit_em2(k, rus, nu):
            # scan-layout em: stationary = 32 strided cols {s*16*nu + c*nu + u'}
            # -> out partitions 32k + s*16 + c == pi directly (no DRAM trip)
            for up in range(nu):
                for ho in range(2):
                    nc.tensor.matmul(
                        _apf(em2p, up * K, [[1, K]], nparts=32, p0=32 * k),
                        _ap(rus[ho], up, [[rus[ho].ap[0][0], 128],
                                          [16 * nu, 2], [nu, 16]]),
                        lt_sb[:, ho, :],
                        start=(ho == 0), stop=(ho == 1),
                        tile_position=(0, 32 * k),
                    )

        def emit_linear(si, rus, q0, nq, via_act=False):
            """linear + em copy + em write for (seq si, slab)."""
            eps = emp.tile([128, nq, K], F32, tag="eps")
            for q in range(nq):
                for ho in range(2):
                    nc.tensor.matmul(
                        eps[:, q, :],
                        rus[ho][:, q * 128:(q + 1) * 128],
                        lt_sb[:, ho, :],
                        start=(ho == 0), stop=(ho == 1),
                    )
            em_sb = empool.tile([128, nq, K], F32, tag="esb")
            nc.scalar.copy(out=em_sb[:, :, :], in_=eps[:, :, :])
            # dram_em [s, qg, p_slice, j]: addr = s*4096 + qg*512 + p*4 + j
            eng = nc.scalar if via_act else nc.sync
            eng.dma_start(
                out=_ap(dram_em, si * 4096 + q0 * 512,
                        [[K, 128], [512, nq], [1, K]]),
                in_=em_sb[:, :, :],
            )

        def emit_gather(slab):
            u0, nu, q0, nq, cpq = SLABS[slab]
            # out: scan_em[:, u0:u0+nu, :] ; partition map pi = b*16 + c
            # in: iter (b, q, cl, u, j) -> addr = b*4096 + (q0+q)*512
            #     + (cl*nu+u)*4 + j ; (q,cl,u,j) collapses to one contiguous
            #     run of nq*512 els per b.
            nc.sync.dma_start(
                out=_ap(scan_em, u0 * K, [[sp_, 128], [1, nu * K]]),
                in_=_ap(dram_em, q0 * 512, [[4096, BPC], [1, nq * 512]]),
            )

        def emit_tree(slab):
            u0, nu, q0, nq, cpq = SLABS[slab]
            if slab < 2:
                # M build for this u range (slab 2's M is deferred off the
                # critical path; its lvl1 is fused with the em directly)
                nc.vector.tensor_tensor(
                    out=_apf(M, u0 * 16, [[16, nu], [4, 4], [1, 4]]),
                    in0=_apf(scan_em, u0 * K, [[K, nu], [0, 4], [1, 4]]),
                    in1=_apf(trans_sb, 0, [[0, nu], [4, 4], [1, 4]]),
                    op=mybir.AluOpType.add,
                )
            if slab == 0:
                # chunk-0 u=0: M_0 = identity + lin_b + em_0. e0fix is
                # (e_mat - crf_trans) on chunk-0 partitions (p % 16 == 0),
                # 0 elsewhere, so a full-width add fixes only chunk 0.
                nc.vector.tensor_tensor(
                    out=_apf(M, 0, [[4, 4], [1, 4]]),
                    in0=_apf(M, 0, [[4, 4], [1, 4]]),
                    in1=_apf(e0fix_sb, 0, [[4, 4], [1, 4]]),
                    op=mybir.AluOpType.add,
                )
                tree_level(1, 0, 16)
                tree_level(2, 0, 8)
                tree_level(3, 0, 4)
                tree_level(4, 0, 2)
                tree_level(5, 0, 1)
            elif slab == 1:
                tree_level(1, 16, 8)
                tree_level(2, 8, 4)
                tree_level(3, 4, 2)
                tree_level(4, 2, 1)
                # P = u0..47 product = Uf5[n0] (x) Uf4[n2]  (hidden under phase 1)
                product(Gp, 0, Uf[5], 0, Uf[4], 2 * 16)
            else:
                # fused lvl1: tmp[pair,x,y] = M[2v][i,y] + M[2v+1][y,x]
                #   = trans2[i,x,y] + (em[2v][y] + em[2v+1][x])
                # so the M build drops off the critical path (only the
                # down-sweeps need M; it is built later, in the G-trip idle).
                em2t = tpool.tile([128, 8, 4, 4], F32, tag="em2t")
                nc.vector.tensor_tensor(
                    out=em2t[:, :, :, :],
                    in0=_apf(scan_em, 48 * K, [[8, 8], [0, 4], [1, 4]]),
                    in1=_apf(scan_em, 48 * K + K, [[8, 8], [1, 4], [0, 4]]),
                    op=mybir.AluOpType.add,
                )
                tmp1 = tpool.tile([128, 8 * 64], F32, tag="tmp1f")
                for i in range(4):
                    nc.vector.tensor_tensor(
                        out=_apf(tmp1, i * 16, [[64, 8], [4, 4], [1, 4]]),
                        in0=_apf(em2t, 0, [[16, 8], [4, 4], [1, 4]]),
                        in1=_apf(trans2_sb, i * 16, [[0, 8], [4, 4], [1, 4]]),
                        op=mybir.AluOpType.add,
                    )
                nc.vector.tensor_reduce(
                    out=_apf(Uf[1], 24 * 16, [[16, 8], [4, 4], [1, 4]]),
                    in_=_apf(tmp1, 0, [[64, 8], [16, 4], [4, 4], [1, 4]]),
                    axis=mybir.AxisListType.X, op=mybir.AluOpType.max,
                )
                tree_level(2, 12, 4)
                tree_level(3, 6, 2)
                tree_level(4, 3, 1)
                # G shortcut: depth 4+product on the critical path, written
                # straight into GGT slot 0
                product(GGT, 0, Gp, 0, Uf[4], 3 * 16)

        # ---- phase 1: slab-outer, seq-inner; linear deferred one unit ----
        pending = None  # (si, rus, slab)
        for slab in range(2):
            u0, nu, q0, nq, cpq = SLABS[slab]
            for si in range(BPC):
                rus = emit_conv(si, x_tiles[si // 2], u0, nu)
                if pending is not None:
                    psi, prus, pslab = pending
                    pu0, pnu, pq0, pnq, pcpq = SLABS[pslab]
                    emit_linear(psi, prus, pq0, pnq)
                    if psi == BPC - 1:
                        emit_gather(pslab)
                        emit_tree(pslab)
                pending = (si, rus, slab)
        # slab 2 (u48-63): seq-paired conv, em placed in scan layout by PE
        u0, nu = 48, 16
        prev_pair = None
        for k in range(4):
            rusp = emit_conv_pair(k, u0, nu)
            if k == 0:
                psi, prus, pslab = pending
                emit_linear(psi, prus, SLABS[1][2], SLABS[1][3])
                emit_gather(1)
                emit_tree(1)
                pending = None
            if prev_pair is not None:
                emit_em2(prev_pair[0], prev_pair[1], nu)
            prev_pair = (k, rusp)
        emit_em2(prev_pair[0], prev_pair[1], nu)
        # one PSUM->SBUF copy lands all of slab 2's em in scan layout
        nc.scalar.copy(out=_apf(scan_em, 48 * K, [[1, 16 * K]]),
                       in_=em2p[:, :, :])
        emit_tree(2)

        # ---- G transpose trip ----
        nc.vector.tensor_copy(out=GGT[:, 1, :], in_=_apf(GGT, 0, [[1, 4], [4, 4]]))
        # off-path work filling the G-trip idle: slab2's M (down-sweeps read
        # it at d=5) and Uf5[n1] (beta d=0)
        nc.vector.tensor_tensor(
            out=_apf(M, 48 * 16, [[16, 16], [4, 4], [1, 4]]),
            in0=_apf(scan_em, 48 * K, [[K, 16], [0, 4], [1, 4]]),
            in1=_apf(trans_sb, 0, [[0, 16], [4, 4], [1, 4]]),
            op=mybir.AluOpType.add,
        )
        tree_level(5, 1, 1)
        # write: dram_g[p, fb, e] contiguous (p = b*16 + c)
        nc.sync.dma_start(
            out=_ap(dram_g, 0, [[32, 128], [1, 32]]),
            in_=GGT[:, :, :],
        )
        # dup gather: gm[p', c'', fb, e] = dram_g[b(p')*16 + c'', fb, e]
        # iter (b, c-dup, c'', fb, e): [[512, 8], [0, 16], [1, 512]]
        gm = spool.tile([128, NCH, 2, 16], F32, tag="gm")
        nc.sync.dma_start(
            out=gm[:, :, :, :],
            in_=_ap(dram_g, 0, [[512, 8], [0, 16], [1, 512]]),
        )

        # ---- full-width fused mid chains ----
        stfb = spool.tile([128, NCH, 2, K], F32, tag="stfb")
        nc.vector.tensor_copy(out=stfb[:, 0, 0, :], in_=start_sb[:, :])
        nc.vector.tensor_copy(out=stfb[:, 0, 1, :], in_=end_sb[:, :])
        for i in range(1, NCH):
            tmpm = mpool.tile([128, 2, K, K], F32, tag="mtm")
            # gm free layout (c', fb, e): fwd -> gm[i-1, 0, :] at (i-1)*32;
            # bwd -> gm[16-i, 1, :] at (16-i)*32 + 16
            fbs = (16 - i) * 32 + 16 - (i - 1) * 32
            nc.vector.tensor_tensor(
                out=tmpm[:, :, :, :],
                in0=_apf(stfb, (i - 1) * 2 * K, [[K, 2], [0, 4], [1, 4]]),
                in1=_apf(gm, (i - 1) * 32, [[fbs, 2], [1, 4], [4, 4]]),
                op=mybir.AluOpType.add,
            )
            nc.vector.tensor_reduce(out=stfb[:, i, :, :], in_=tmpm[:, :, :, :],
                                    axis=mybir.AxisListType.X,
                                    op=mybir.AluOpType.max)

        # ---- local seed select (host masks; no DRAM trip) ----
        sv = spool.tile([128, CL + 1, K], F32, tag="sv")
        wv = spool.tile([128, CL, K], F32, tag="wv")
        selt = mpool.tile([128, K, NCH], F32, tag="selt")
        # alpha seed: sv[:,0,j] = sum_c stfb[:,c,0,j] * emask[:,c]
        nc.vector.tensor_tensor(
            out=selt[:, :, :],
            in0=_apf(stfb, 0, [[1, 4], [8, 16]]),
            in1=_apf(emask_sb, 0, [[0, 4], [1, 16]]),
            op=mybir.AluOpType.mult,
        )
        nc.vector.tensor_reduce(out=sv[:, 0, :], in_=selt[:, :, :],
                                axis=mybir.AxisListType.X, op=mybir.AluOpType.add)
        # beta seed: wv[:,CL-1,j] = sum_c stfb[:,c,1,j] * wmask[:,c]
        selt2 = mpool.tile([128, K, NCH], F32, tag="selt2")
        nc.vector.tensor_tensor(
            out=selt2[:, :, :],
            in0=_apf(stfb, K, [[1, 4], [8, 16]]),
            in1=_apf(wmask_sb, 0, [[0, 4], [1, 16]]),
            op=mybir.AluOpType.mult,
        )
        nc.vector.tensor_reduce(out=wv[:, CL - 1, :], in_=selt2[:, :, :],
                                axis=mybir.AxisListType.X, op=mybir.AluOpType.add)

        # ---- down-sweeps, alpha/beta interleaved ----
        t64 = tpool.tile([128, 4, 4], F32, tag="t64")
        nc.vector.tensor_tensor(
            out=t64[:, :, :],
            in0=_apf(sv, 0, [[0, 4], [1, 4]]),
            in1=_apf(GGT, 16, [[4, 4], [1, 4]]),
            op=mybir.AluOpType.add,
        )
        nc.vector.tensor_reduce(out=sv[:, CL, :], in_=t64[:, :, :],
                                axis=mybir.AxisListType.X, op=mybir.AluOpType.max)
        for d in range(6):
            s_ = CL >> d
            n = 1 << d
            tmpa = tpool.tile([128, n, 4, 4], F32, tag="tmpa")
            nc.vector.tensor_tensor(
                out=tmpa[:, :, :, :],
                in0=_apf(sv, 0, [[s_ * 4, n], [0, 4], [1, 4]]),
                in1=_apf(Uf[5 - d], 0, [[32, n], [1, 4], [4, 4]]),
                op=mybir.AluOpType.add,
            )
            nc.vector.tensor_reduce(
                out=_apf(sv, (s_ // 2) * 4, [[s_ * 4, n], [1, 4]]),
                in_=tmpa[:, :, :, :],
                axis=mybir.AxisListType.X, op=mybir.AluOpType.max,
            )
            tmpb = tpool.tile([128, n, 4, 4], F32, tag="tmpb")
            nc.vector.tensor_tensor(
                out=tmpb[:, :, :, :],
                in0=_apf(wv, (s_ - 1) * 4, [[s_ * 4, n], [0, 4], [1, 4]]),
                in1=_apf(Uf[5 - d], 16, [[32, n], [4, 4], [1, 4]]),
                op=mybir.AluOpType.add,
            )
            nc.vector.tensor_reduce(
                out=_apf(wv, (s_ // 2 - 1) * 4, [[s_ * 4, n], [1, 4]]),
                in_=tmpb[:, :, :, :],
                axis=mybir.AxisListType.X, op=mybir.AluOpType.max,
            )

        # ---- tags: argmax_j(alpha + beta), ties -> smallest j ----
        t4 = spool.tile([128, CL, K], F32, tag="t4")
        nc.vector.tensor_tensor(out=t4[:, :, :],
                                in0=_apf(sv, K, [[1, CL * K]]),
                                in1=_apf(wv, 0, [[1, CL * K]]),
                                op=mybir.AluOpType.add)
        rm = spool.tile([128, CL], F32, tag="rm")
        nc.vector.tensor_reduce(out=rm[:, :], in_=t4[:, :, :],
                                axis=mybir.AxisListType.X, op=mybir.AluOpType.max)
        eq = spool.tile([128, CL, K], F32, tag="eq")
        nc.vector.tensor_tensor(out=eq[:, :, :], in0=t4[:, :, :],
                                in1=_apf(rm, 0, [[1, CL], [0, K]]),
                                op=mybir.AluOpType.is_lt)
        t5 = spool.tile([128, CL, K], F32, tag="t5")
        nc.vector.scalar_tensor_tensor(
            out=t5[:, :, :], in0=eq[:, :, :], scalar=BIG,
            in1=_apf(iota_sb, 0, [[0, CL], [1, K]]),
            op0=mybir.AluOpType.mult, op1=mybir.AluOpType.add)
        amn = spool.tile([128, CL], F32, tag="amn")
        nc.vector.tensor_reduce(out=amn[:, :], in_=t5[:, :, :],
                                axis=mybir.AxisListType.X, op=mybir.AluOpType.min)
        tagi = spool.tile([128, CL], I32, tag="tagi")
        nc.vector.tensor_copy(out=tagi[:, :], in_=amn[:, :])
        # pi = b*16 + c: out addr = p*64 + u (fully contiguous)
        nc.sync.dma_start(
            out=_ap(out_tags, 0, [[CL, 128], [1, CL]]),
            in_=tagi[:, :],
        )

    if split_waits:
        _split_multi_waits(nc)
    return nc


def prep_core_inputs(core, sentence_features, conv_w, conv_b, lin_w, lin_b,
                     crf_start, crf_end, crf_trans):
    sf = np.asarray(sentence_features, np.float32)
    conv_w = np.asarray(conv_w, np.float32)
    conv_b = np.asarray(conv_b, np.float32)
    lin_w = np.asarray(lin_w, np.float32)
    lin_b = np.asarray(lin_b, np.float32)
    crf_start = np.asarray(crf_start, np.float32)
    crf_end = np.asarray(crf_end, np.float32)
    crf_trans = np.asarray(crf_trans, np.float32)

    xsh = sf[core * BPC:(core + 1) * BPC]  # [8, T, H]
    xpad = np.zeros((BPC, H, T + 2), np.float32)
    xpad[:, :, 1:T + 1] = xsh.transpose(0, 2, 1)
    xp = np.ascontiguousarray(xpad.reshape(BPC, 2, 128, T + 2)).astype(np.float16)

    wt = conv_w.transpose(1, 0, 2)  # [hin, hout, k]
    wcv = np.empty((12, 128, 128), np.float32)
    for ho in range(2):
        for k in range(3):
            for hi in range(2):
                g = ho * 6 + k * 2 + hi
                wcv[g] = wt[hi * 128:(hi + 1) * 128, ho * 128:(ho + 1) * 128, k]

    transp = crf_trans + lin_b[None, :]
    trans2 = np.empty((4, 4, 4), np.float32)
    for i in range(4):
        for x in range(4):
            for y in range(4):
                trans2[i, x, y] = transp[i, y] + transp[y, x]
    e_mat = np.full((4, 4), NEG, np.float32)
    np.fill_diagonal(e_mat, 0.0)
    # partition map pi = b*16 + c: c = p % 16
    cvec = np.arange(128) % 16
    emask = np.zeros((128, 16), np.float32)
    emask[np.arange(128), cvec] = 1.0
    wmask = np.zeros((128, 16), np.float32)
    wmask[np.arange(128), 15 - cvec] = 1.0
    e0fix = np.zeros((128, 16), np.float32)
    e0fix[cvec == 0] = (e_mat - crf_trans).reshape(16)

    return {
        "xp": xp,
        "wcv": wcv.astype(np.float16),
        "lt": np.ascontiguousarray(lin_w.T.reshape(2, 128, K)).astype(np.float16),
        "cb": conv_b.reshape(2, 128).copy(),
        "trans_r": np.tile(transp.reshape(1, 16), (128, 1)).copy(),
        "trans2_r": np.tile(trans2.reshape(1, 64), (128, 1)).copy(),
        "e0fix_r": e0fix,
        "iota_r": np.tile(np.arange(4, dtype=np.float32), (128, 1)).copy(),
        "emask_r": emask,
        "wmask_r": wmask,
        "start_r": np.tile(crf_start[None, :], (128, 1)).astype(np.float32),
        "end_r": np.tile(crf_end[None, :], (128, 1)).astype(np.float32),
    }


_NC_CACHE = None


def kernel(sentence_features, conv_w, conv_b, lin_w, lin_b, crf_start,
           crf_end, crf_trans):
    global _NC_CACHE
    if _NC_CACHE is None:
        _NC_CACHE = build_program()
    nc = _NC_CACHE
    in_maps = [
        prep_core_inputs(c, sentence_features, conv_w, conv_b, lin_w, lin_b,
                         crf_start, crf_end, crf_trans)
        for c in range(NCORES)
    ]
    res = bass_utils.run_bass_kernel_spmd(nc, in_maps, core_ids=list(range(NCORES)))
    kernel.last_results = res
    out = np.concatenate([res.results[c]["out_tags"] for c in range(NCORES)], axis=0)
    return out.astype(np.int32)


# revision 10
# speedup vs baseline: 1.0372x; 1.0013x over previous
"""Trainium2 Bass kernel v3: Conv1d(256,256,k=3) -> ReLU -> Linear(256,4) -> CRF Viterbi.

8 cores, data-parallel over batch (8 sequences/core). 81.9us vs v2's 105.1us
(TimelineSim; same cost model as the harness fallback).

Emissions (PE + Act):
  - Conv: fp16 1-term matmuls, (chunk,u)-major psum layout via 2-free-dim
    moving APs, split into u-slabs (u0-31 / u32-47 / u48-63) so most of the
    Viterbi up-sweep can run under the conv.
  - Reversed linear: stationary = relu tile slice [128h, 128t], moving = lt
    [128h, 4]; out psum [128 t-local, 4]. PE cost ~0.2us vs 6.8us for the
    stationary=lt orientation (matmul cost ~ out free size; Ldweights free).
  - Warm-up matmuls on a memset scratch tile (no DMA dependency) ramp the
    PE p-state from ~1.3us; startup-critical input DMAs are split
    (w ho-halves, xt0 hi-halves) so conv starts ~4.4us.

em staging: Act copies psum->SBUF, per-(seq,slab) DMA to a p-flat DRAM
layout [s, q, p_slice, j], then one gather per slab into scan_em
[p = b*16+c, u, j]. The b-major partition map makes both bounce sides
collapse to <=3 AP dims (hardware DMA limit) with 512B runs. em writes go
out on SP; the last one on Act's HWDGE queue to dodge serialization.

Viterbi decode (DVE only -- Pool/gpsimd cannot execute vector ops on HW):
  - M_t[m,j] = trans'[m,j] + em_t[j]; chunk-0 u=0 fixed to (maxplus identity
    + lin_b + em_0) via a host mask add, so G_0 absorbs em_0 and the chain
    init is a replicated constant.
  - Blelloch up-sweep per 64-position chunk; slabs u<48 hidden under conv;
    post-conv remainder is depth-4 plus G = P (x) Uf4[n3] with P = u0..47
    product precomputed under phase 1.
  - G transpose trip: one contiguous write [p, fb, e] + one 16x-duplicating
    gather (0-stride DRAM dim) gives every partition its seq's 16 chunk
    products; fused fwd/bwd mid chains then run full-width, and
    host-precomputed masks select each partition's alpha/beta seed locally
    (no seeds DRAM round trip).
  - Interleaved alpha/beta down-sweeps; tags = argmax_j(alpha+beta) with an
    is_lt/iota trick (ties -> smallest j, matching reference backtracking).
  - pi = b*16+c makes the output DMA fully contiguous.
"""

import numpy as np

import concourse.bass as bass
import concourse.tile as tile
from concourse import mybir
from concourse import bass_utils

B, T, H, K = 64, 1024, 256, 4
NCORES = 8
BPC = B // NCORES
NCH = 16
CL = 64
NEG = -1.0e30
BIG = 1024.0
F32 = mybir.dt.float32
F16 = mybir.dt.float16
I32 = mybir.dt.int32
DUMMY_MMS = 8

# slabs: (u0, nu, q0, nq, cpq)  cpq = chunks per 128-col slice
SLABS = [(0, 32, 0, 4, 4), (32, 16, 4, 2, 8), (48, 16, 6, 2, 8)]


def _ap(t, off, pairs):
    if hasattr(t, "tensor"):
        return bass.AP(tensor=t.tensor, offset=t.offset + off, ap=pairs)
    return bass.AP(tensor=t, offset=off, ap=pairs)


def _apf(t, off, pairs, nparts=None, p0=0):
    p = list(t.ap[0]) if nparts is None else [t.ap[0][0], nparts]
    return bass.AP(tensor=t.tensor, offset=t.offset + p0 * t.ap[0][0] + off,
                   ap=[p] + pairs)


def _split_multi_waits(nc):
    """Walrus allows one semaphore wait per instruction; split extras onto
    same-engine NoOps inserted just before."""
    ctr = 0
    for fn in nc.m.functions:
        for blk in fn.blocks:
            insts = list(blk.instructions)
            new = []
            changed = False
            for inst in insts:
                si = inst.sync_info
                if si is not None and len(si.on_wait) > 1:
                    waits = list(si.on_wait)
                    for w in waits[:-1]:
                        nop = mybir.InstNoOp(name=f"I-ws-{ctr}", ins=[], outs=[])
                        ctr += 1
                        nop.engine = inst.engine
                        nop.sync_info = mybir.SyncInfo(on_wait=[w], on_update=[])
                        new.append(nop)
                    inst.sync_info = mybir.SyncInfo(
                        on_wait=[waits[-1]], on_update=list(si.on_update))
                    changed = True
                new.append(inst)
            if changed:
                blk.instructions = new
    return ctr


def build_program(split_waits=True):
    nc = bass.Bass("TRN2", debug=False, num_devices=NCORES)

    xp = nc.dram_tensor("xp", [BPC, 2, 128, T + 2], F16, kind="ExternalInput")
    wcv = nc.dram_tensor("wcv", [12, 128, 128], F16, kind="ExternalInput")
    lt = nc.dram_tensor("lt", [2, 128, K], F16, kind="ExternalInput")
    cb = nc.dram_tensor("cb", [2, 128], F32, kind="ExternalInput")
    trans_r = nc.dram_tensor("trans_r", [128, 16], F32, kind="ExternalInput")
    trans2_r = nc.dram_tensor("trans2_r", [128, 64], F32, kind="ExternalInput")
    e0fix_r = nc.dram_tensor("e0fix_r", [128, 16], F32, kind="ExternalInput")
    iota_r = nc.dram_tensor("iota_r", [128, K], F32, kind="ExternalInput")
    emask_r = nc.dram_tensor("emask_r", [128, 16], F32, kind="ExternalInput")
    wmask_r = nc.dram_tensor("wmask_r", [128, 16], F32, kind="ExternalInput")
    start_r = nc.dram_tensor("start_r", [128, K], F32, kind="ExternalInput")
    end_r = nc.dram_tensor("end_r", [128, K], F32, kind="ExternalInput")
    out_tags = nc.dram_tensor("out_tags", [BPC, T], I32, kind="ExternalOutput")

    from contextlib import ExitStack
    with tile.TileContext(nc) as tc, ExitStack() as ctx:
        consts = ctx.enter_context(tc.tile_pool(name="consts", bufs=1))
        xpool = ctx.enter_context(tc.tile_pool(name="xpool", bufs=8))
        convp = ctx.enter_context(tc.tile_pool(name="convp", bufs=3, space="PSUM"))
        emp = ctx.enter_context(tc.tile_pool(name="emp", bufs=1, space="PSUM"))
        em2pool = ctx.enter_context(tc.tile_pool(name="em2pool", bufs=1, space="PSUM"))
        relup = ctx.enter_context(tc.tile_pool(name="relup", bufs=10))
        empool = ctx.enter_context(tc.tile_pool(name="empool", bufs=4))
        spool = ctx.enter_context(tc.tile_pool(name="spool", bufs=1))
        tpool = ctx.enter_context(tc.tile_pool(name="tpool", bufs=4))
        mpool = ctx.enter_context(tc.tile_pool(name="mpool", bufs=3))
        dpool = ctx.enter_context(tc.tile_pool(name="dpool", bufs=1, space="DRAM"))

        # ---- leading DMAs ----
        # warm-up scratch: initialized by a cheap Pool memset (no DMA), so
        # the PE p-state ramp starts almost immediately
        wd_sb = consts.tile([128, 128], F16, tag="wd")
        nc.gpsimd.memset(wd_sb[:, :], 0.0)
        x_tiles = []
        SEQB = 2 * 128 * (T + 2)  # xp seq stride (els)
        xt0 = xpool.tile([128, 2, 2, T + 2], F16, tag="xt", name="xt0")
        w_sb = consts.tile([128, 12, 128], F16, tag="w")
        # startup-critical splits: ho0 conv weights, then seq0's hi0 half
        nc.sync.dma_start(out=w_sb[:, 0:6, :],
                          in_=_ap(wcv, 0, [[128, 128], [16384, 6], [1, 128]]))
        nc.sync.dma_start(out=_apf(xt0, 0, [[1, T + 2]]),
                          in_=_ap(xp, 0, [[T + 2, 128], [1, T + 2]]))
        nc.sync.dma_start(out=_apf(xt0, T + 2, [[1, T + 2]]),
                          in_=_ap(xp, 128 * (T + 2), [[T + 2, 128], [1, T + 2]]))
        nc.sync.dma_start(out=_apf(xt0, 2 * (T + 2), [[1, 2 * (T + 2)]]),
                          in_=_ap(xp, SEQB, [[T + 2, 128], [SEQB // 2, 2],
                                             [1, T + 2]]))
        nc.sync.dma_start(out=w_sb[:, 6:12, :],
                          in_=_ap(wcv, 6 * 128 * 128, [[128, 128], [16384, 6], [1, 128]]))
        x_tiles.append(xt0)
        lt_sb = consts.tile([128, 2, K], F16, tag="lt")
        nc.sync.dma_start(out=lt_sb[:, :, :], in_=lt.ap().rearrange("h p j -> p h j"))
        cb_sb = consts.tile([128, 2], F32, tag="cb")
        nc.sync.dma_start(out=cb_sb[:, :], in_=cb.ap().rearrange("h p -> p h"))
        for kk in range(1, 4):
            xt = xpool.tile([128, 2, 2, T + 2], F16, tag="xt", name=f"xt{kk}")
            nc.sync.dma_start(
                out=xt[:, :, :, :],
                in_=_ap(xp, 2 * kk * SEQB, [[T + 2, 128], [SEQB // 2, 4],
                                            [1, T + 2]]))
            x_tiles.append(xt)
        trans_sb = consts.tile([128, 16], F32, tag="tr")
        nc.sync.dma_start(out=trans_sb[:, :], in_=trans_r.ap())
        trans2_sb = consts.tile([128, 4, 16], F32, tag="tr2")
        nc.sync.dma_start(out=trans2_sb[:, :, :], in_=trans2_r.ap())
        e0fix_sb = consts.tile([128, 16], F32, tag="e0f")
        nc.sync.dma_start(out=e0fix_sb[:, :], in_=e0fix_r.ap())
        iota_sb = consts.tile([128, K], F32, tag="io")
        nc.sync.dma_start(out=iota_sb[:, :], in_=iota_r.ap())
        emask_sb = consts.tile([128, 16], F32, tag="em")
        nc.sync.dma_start(out=emask_sb[:, :], in_=emask_r.ap())
        wmask_sb = consts.tile([128, 16], F32, tag="wm")
        nc.sync.dma_start(out=wmask_sb[:, :], in_=wmask_r.ap())
        start_sb = consts.tile([128, K], F32, tag="sst")
        nc.sync.dma_start(out=start_sb[:, :], in_=start_r.ap())
        end_sb = consts.tile([128, K], F32, tag="send")
        nc.sync.dma_start(out=end_sb[:, :], in_=end_r.ap())

        dram_em = dpool.tile([BPC, 8, 128, K], F32, tag="dem")
        dram_g = dpool.tile([128, 2, 16], F32, tag="dg")

        # ---- persistent scan tiles ----
        scan_em = spool.tile([128, CL, K], F32, tag="sem")  # [p=(b,c), u, j]
        em2p = em2pool.tile([128, 16, K], F32, tag="em2p")
        sp_ = scan_em.ap[0][0]
        M = spool.tile([128, CL, K, K], F32, tag="M")
        Uf = [M]
        for lvl in range(1, 7):
            n = CL >> lvl
            Uf.append(spool.tile([128, n * 16], F32, tag=f"Uf{lvl}",
                                 name=f"Uf{lvl}"))
        G = Uf[6]
        Gp = spool.tile([128, 16], F32, tag="Gp")
        GGT = spool.tile([128, 2, 16], F32, tag="GGT")
        Gp2 = spool.tile([128, 16], F32, tag="Gp2")

        # ---- PE warm-up ----
        warm = convp.tile([128, 128], F32, tag="warm")
        for _ in range(DUMMY_MMS):
            nc.tensor.matmul(warm[:, :], wd_sb[:, :], wd_sb[:, :],
                             start=True, stop=True)

        def product(dst, d_off, srcA, a_off, srcB, b_off, use_pool=False):
            """dst[d_off] = srcA[a_off] (x) srcB[b_off], single 4x4 node."""
            tmp = tpool.tile([128, 64], F32, tag="tmp1")
            for i in range(4):
                eng = nc.gpsimd if (use_pool and i >= 2) else nc.vector
                eng.tensor_tensor(
                    out=_apf(tmp, i * 16, [[64, 1], [4, 4], [1, 4]]),
                    in0=_apf(srcA, a_off + i * 4, [[32, 1], [0, 4], [1, 4]]),
                    in1=_apf(srcB, b_off, [[32, 1], [1, 4], [4, 4]]),
                    op=mybir.AluOpType.add,
                )
            nc.vector.tensor_reduce(
                out=_apf(dst, d_off, [[16, 1], [4, 4], [1, 4]]),
                in_=_apf(tmp, 0, [[64, 1], [16, 4], [4, 4], [1, 4]]),
                axis=mybir.AxisListType.X, op=mybir.AluOpType.max,
            )

        def tree_level(lvl, v0, n, use_pool=False):
            prev = Uf[lvl - 1]
            tmp = tpool.tile([128, n * 64], F32, tag="tmp")
            for i in range(4):
                eng = nc.gpsimd if (use_pool and i >= 2) else nc.vector
                eng.tensor_tensor(
                    out=_apf(tmp, i * 16, [[64, n], [4, 4], [1, 4]]),
                    in0=_apf(prev, v0 * 32 + i * 4, [[32, n], [0, 4], [1, 4]]),
                    in1=_apf(prev, v0 * 32 + 16, [[32, n], [1, 4], [4, 4]]),
                    op=mybir.AluOpType.add,
                )
            nc.vector.tensor_reduce(
                out=_apf(Uf[lvl], v0 * 16, [[16, n], [4, 4], [1, 4]]),
                in_=_apf(tmp, 0, [[64, n], [16, 4], [4, 4], [1, 4]]),
                axis=mybir.AxisListType.X, op=mybir.AluOpType.max,
            )

        def emit_conv(si, xt, u0, nu):
            soff = (si % 2) * 2 * (T + 2)
            """conv for (seq si, slab): returns relu tiles per ho."""
            rus = {}
            for ho in range(2):
                cp = convp.tile([128, 16 * nu], F32, tag="cp")
                for i in range(6):
                    hi, k = i // 3, i % 3
                    g = ho * 6 + k * 2 + hi
                    nc.tensor.matmul(
                        cp[:, :],
                        w_sb[:, g, :],
                        _ap(xt, soff + hi * (T + 2) + k + u0,
                            [[xt.ap[0][0], 128], [64, 16], [1, nu]]),
                        start=(i == 0), stop=(i == 5),
                    )
                ru = relup.tile([128, 16 * nu], F16, tag="ru")
                nc.scalar.activation(
                    out=ru[:, :], in_=cp[:, :],
                    func=mybir.ActivationFunctionType.Relu,
                    bias=cb_sb[:, ho:ho + 1], scale=1.0,
                )
                rus[ho] = ru
            return rus

        def emit_conv_pair(k, u0, nu):
            # both seqs of pair k in one matmul set: psum cols (s, c, u)
            xt = x_tiles[k]
            rus = {}
            for ho in range(2):
                cp = convp.tile([128, 32 * nu], F32, tag="cp")
                for i in range(6):
                    hi, kk = i // 3, i % 3
                    g = ho * 6 + kk * 2 + hi
                    nc.tensor.matmul(
                        cp[:, :],
                        w_sb[:, g, :],
                        _ap(xt, hi * (T + 2) + kk + u0,
                            [[xt.ap[0][0], 128], [2 * (T + 2), 2],
                             [64, 16], [1, nu]]),
                        start=(i == 0), stop=(i == 5),
                    )
                ru = relup.tile([128, 32 * nu], F16, tag="ru")
                nc.scalar.activation(
                    out=ru[:, :], in_=cp[:, :],
                    func=mybir.ActivationFunctionType.Relu,
                    bias=cb_sb[:, ho:ho + 1], scale=1.0,
                )
                rus[ho] = ru
            return rus

        def emit_em2(k, rus, nu):
            # scan-layout em: stationary = 32 strided cols {s*16*nu + c*nu + u'}
            # -> out partitions 32k + s*16 + c == pi directly (no DRAM trip)
            for up in range(nu):
                for ho in range(2):
                    nc.tensor.matmul(
                        _apf(em2p, up * K, [[1, K]], nparts=32, p0=32 * k),
                        _ap(rus[ho], up, [[rus[ho].ap[0][0], 128],
                                          [16 * nu, 2], [nu, 16]]),
                        lt_sb[:, ho, :],
                        start=(ho == 0), stop=(ho == 1),
                        tile_position=(0, 32 * k),
                    )

        def emit_linear(si, rus, q0, nq, via_act=False):
            """linear + em copy + em write for (seq si, slab)."""
            eps = emp.tile([128, nq, K], F32, tag="eps")
            for q in range(nq):
                for ho in range(2):
                    nc.tensor.matmul(
                        eps[:, q, :],
                        rus[ho][:, q * 128:(q + 1) * 128],
                        lt_sb[:, ho, :],
                        start=(ho == 0), stop=(ho == 1),
                    )
            em_sb = empool.tile([128, nq, K], F32, tag="esb")
            nc.scalar.copy(out=em_sb[:, :, :], in_=eps[:, :, :])
            # dram_em [s, qg, p_slice, j]: addr = s*4096 + qg*512 + p*4 + j
            eng = nc.scalar if via_act else nc.sync
            eng.dma_start(
                out=_ap(dram_em, si * 4096 + q0 * 512,
                        [[K, 128], [512, nq], [1, K]]),
                in_=em_sb[:, :, :],
            )

        def emit_gather(slab):
            u0, nu, q0, nq, cpq = SLABS[slab]
            # out: scan_em[:, u0:u0+nu, :] ; partition map pi = b*16 + c
            # in: iter (b, q, cl, u, j) -> addr = b*4096 + (q0+q)*512
            #     + (cl*nu+u)*4 + j ; (q,cl,u,j) collapses to one contiguous
            #     run of nq*512 els per b.
            nc.sync.dma_start(
                out=_ap(scan_em, u0 * K, [[sp_, 128], [1, nu * K]]),
                in_=_ap(dram_em, q0 * 512, [[4096, BPC], [1, nq * 512]]),
            )

        def emit_tree(slab):
            u0, nu, q0, nq, cpq = SLABS[slab]
            if slab < 2:
                # M build for this u range (slab 2's M is deferred off the
                # critical path; its lvl1 is fused with the em directly)
                nc.vector.tensor_tensor(
                    out=_apf(M, u0 * 16, [[16, nu], [4, 4], [1, 4]]),
                    in0=_apf(scan_em, u0 * K, [[K, nu], [0, 4], [1, 4]]),
                    in1=_apf(trans_sb, 0, [[0, nu], [4, 4], [1, 4]]),
                    op=mybir.AluOpType.add,
                )
            if slab == 0:
                # chunk-0 u=0: M_0 = identity + lin_b + em_0. e0fix is
                # (e_mat - crf_trans) on chunk-0 partitions (p % 16 == 0),
                # 0 elsewhere, so a full-width add fixes only chunk 0.
                nc.vector.tensor_tensor(
                    out=_apf(M, 0, [[4, 4], [1, 4]]),
                    in0=_apf(M, 0, [[4, 4], [1, 4]]),
                    in1=_apf(e0fix_sb, 0, [[4, 4], [1, 4]]),
                    op=mybir.AluOpType.add,
                )
                tree_level(1, 0, 16)
                tree_level(2, 0, 8)
                tree_level(3, 0, 4)
                tree_level(4, 0, 2)
                tree_level(5, 0, 1)
            elif slab == 1:
                tree_level(1, 16, 8)
                tree_level(2, 8, 4)
                tree_level(3, 4, 2)
                tree_level(4, 2, 1)
                # P = u0..47 product = Uf5[n0] (x) Uf4[n2]  (hidden under phase 1)
                product(Gp, 0, Uf[5], 0, Uf[4], 2 * 16)
            else:
                # fused lvl1: tmp[pair,x,y] = M[2v][i,y] + M[2v+1][y,x]
                #   = trans2[i,x,y] + (em[2v][y] + em[2v+1][x])
                # so the M build drops off the critical path (only the
                # down-sweeps need M; it is built later, in the G-trip idle).
                em2t = tpool.tile([128, 8, 4, 4], F32, tag="em2t")
                nc.vector.tensor_tensor(
                    out=em2t[:, :, :, :],
                    in0=_apf(scan_em, 48 * K, [[8, 8], [0, 4], [1, 4]]),
                    in1=_apf(scan_em, 48 * K + K, [[8, 8], [1, 4], [0, 4]]),
                    op=mybir.AluOpType.add,
                )
                tmp1 = tpool.tile([128, 8 * 64], F32, tag="tmp1f")
                for i in range(4):
                    nc.vector.tensor_tensor(
                        out=_apf(tmp1, i * 16, [[64, 8], [4, 4], [1, 4]]),
                        in0=_apf(em2t, 0, [[16, 8], [4, 4], [1, 4]]),
                        in1=_apf(trans2_sb, i * 16, [[0, 8], [4, 4], [1, 4]]),
                        op=mybir.AluOpType.add,
                    )
                nc.vector.tensor_reduce(
                    out=_apf(Uf[1], 24 * 16, [[16, 8], [4, 4], [1, 4]]),
                    in_=_apf(tmp1, 0, [[64, 8], [16, 4], [4, 4], [1, 4]]),
                    axis=mybir.AxisListType.X, op=mybir.AluOpType.max,
                )
                tree_level(2, 12, 4)
                tree_level(3, 6, 2)
                tree_level(4, 3, 1)
                # G shortcut: depth 4+product on the critical path, written
                # straight into GGT slot 0
                product(GGT, 0, Gp, 0, Uf[4], 3 * 16)

        # ---- phase 1: slab-outer, seq-inner; linear deferred one unit ----
        pending = None  # (si, rus, slab)
        for slab in range(2):
            u0, nu, q0, nq, cpq = SLABS[slab]
            for si in range(BPC):
                rus = emit_conv(si, x_tiles[si // 2], u0, nu)
                if pending is not None:
                    psi, prus, pslab = pending
                    pu0, pnu, pq0, pnq, pcpq = SLABS[pslab]
                    emit_linear(psi, prus, pq0, pnq)
                    if psi == BPC - 1:
                        emit_gather(pslab)
                        emit_tree(pslab)
                pending = (si, rus, slab)
        # slab 2 (u48-63): seq-paired conv, em placed in scan layout by PE
        u0, nu = 48, 16
        prev_pair = None
        for k in range(4):
            rusp = emit_conv_pair(k, u0, nu)
            if k == 0:
                psi, prus, pslab = pending
                emit_linear(psi, prus, SLABS[1][2], SLABS[1][3])
                emit_gather(1)
                emit_tree(1)
                pending = None
            if prev_pair is not None:
                emit_em2(prev_pair[0], prev_pair[1], nu)
            prev_pair = (k, rusp)
        emit_em2(prev_pair[0], prev_pair[1], nu)
        # one PSUM->SBUF copy lands all of slab 2's em in scan layout
        nc.scalar.copy(out=_apf(scan_em, 48 * K, [[1, 16 * K]]),
                       in_=em2p[:, :, :])
        emit_tree(2)

        # ---- G transpose trip ----
        nc.vector.tensor_copy(out=GGT[:, 1, :], in_=_apf(GGT, 0, [[1, 4], [4, 4]]))
        # off-path work filling the G-trip idle: slab2's M (down-sweeps read
        # it at d=5) and Uf5[n1] (beta d=0)
        nc.vector.tensor_tensor(
            out=_apf(M, 48 * 16, [[16, 16], [4, 4], [1, 4]]),
            in0=_apf(scan_em, 48 * K, [[K, 16], [0, 4], [1, 4]]),
            in1=_apf(trans_sb, 0, [[0, 16], [4, 4], [1, 4]]),
            op=mybir.AluOpType.add,
        )
        tree_level(5, 1, 1)
        # write: dram_g[p, fb, e] contiguous (p = b*16 + c)
        nc.sync.dma_start(
            out=_ap(dram_g, 0, [[32, 128], [1, 32]]),
            in_=GGT[:, :, :],
        )
        # dup gather: gm[p', c'', fb, e] = dram_g[b(p')*16 + c'', fb, e]
        # iter (b, c-dup, c'', fb, e): [[512, 8], [0, 16], [1, 512]]
        gm = spool.tile([128, NCH, 2, 16], F32, tag="gm")
        nc.sync.dma_start(
            out=gm[:, :, :, :],
            in_=_ap(dram_g, 0, [[512, 8], [0, 16], [1, 512]]),
        )

        # ---- full-width fused mid chains ----
        stfb = spool.tile([128, NCH, 2, K], F32, tag="stfb")
        nc.vector.tensor_copy(out=stfb[:, 0, 0, :], in_=start_sb[:, :])
        nc.vector.tensor_copy(out=stfb[:, 0, 1, :], in_=end_sb[:, :])
        for i in range(1, NCH):
            tmpm = mpool.tile([128, 2, K, K], F32, tag="mtm")
            # gm free layout (c', fb, e): fwd -> gm[i-1, 0, :] at (i-1)*32;
            # bwd -> gm[16-i, 1, :] at (16-i)*32 + 16
            fbs = (16 - i) * 32 + 16 - (i - 1) * 32
            nc.vector.tensor_tensor(
                out=tmpm[:, :, :, :],
                in0=_apf(stfb, (i - 1) * 2 * K, [[K, 2], [0, 4], [1, 4]]),
                in1=_apf(gm, (i - 1) * 32, [[fbs, 2], [1, 4], [4, 4]]),
                op=mybir.AluOpType.add,
            )
            nc.vector.tensor_reduce(out=stfb[:, i, :, :], in_=tmpm[:, :, :, :],
                                    axis=mybir.AxisListType.X,
                                    op=mybir.AluOpType.max)

        # ---- local seed select (host masks; no DRAM trip) ----
        sv = spool.tile([128, CL + 1, K], F32, tag="sv")
        wv = spool.tile([128, CL, K], F32, tag="wv")
        selt = mpool.tile([128, K, NCH], F32, tag="selt")
        # alpha seed: sv[:,0,j] = sum_c stfb[:,c,0,j] * emask[:,c]
        nc.vector.tensor_tensor(
            out=selt[:, :, :],
            in0=_apf(stfb, 0, [[1, 4], [8, 16]]),
            in1=_apf(emask_sb, 0, [[0, 4], [1, 16]]),
            op=mybir.AluOpType.mult,
        )
        nc.vector.tensor_reduce(out=sv[:, 0, :], in_=selt[:, :, :],
                                axis=mybir.AxisListType.X, op=mybir.AluOpType.add)
        # beta seed: wv[:,CL-1,j] = sum_c stfb[:,c,1,j] * wmask[:,c]
        selt2 = mpool.tile([128, K, NCH], F32, tag="selt2")
        nc.vector.tensor_tensor(
            out=selt2[:, :, :],
            in0=_apf(stfb, K, [[1, 4], [8, 16]]),
            in1=_apf(wmask_sb, 0, [[0, 4], [1, 16]]),
            op=mybir.AluOpType.mult,
        )
        nc.vector.tensor_reduce(out=wv[:, CL - 1, :], in_=selt2[:, :, :],
                                axis=mybir.AxisListType.X, op=mybir.AluOpType.add)

        # ---- down-sweeps, alpha/beta interleaved ----
        t64 = tpool.tile([128, 4, 4], F32, tag="t64")
        nc.vector.tensor_tensor(
            out=t64[:, :, :],
            in0=_apf(sv, 0, [[0, 4], [1, 4]]),
            in1=_apf(GGT, 16, [[4, 4], [1, 4]]),
            op=mybir.AluOpType.add,
        )
        nc.vector.tensor_reduce(out=sv[:, CL, :], in_=t64[:, :, :],
                                axis=mybir.AxisListType.X, op=mybir.AluOpType.max)
        for d in range(6):
            s_ = CL >> d
            n = 1 << d
            tmpa = tpool.tile([128, n, 4, 4], F32, tag="tmpa")
            nc.vector.tensor_tensor(
                out=tmpa[:, :, :, :],
                in0=_apf(sv, 0, [[s_ * 4, n], [0, 4], [1, 4]]),
                in1=_apf(Uf[5 - d], 0, [[32, n], [1, 4], [4, 4]]),
                op=mybir.AluOpType.add,
            )
            nc.vector.tensor_reduce(
                out=_apf(sv, (s_ // 2) * 4, [[s_ * 4, n], [1, 4]]),
                in_=tmpa[:, :, :, :],
                axis=mybir.AxisListType.X, op=mybir.AluOpType.max,
            )
            tmpb = tpool.tile([128, n, 4, 4], F32, tag="tmpb")
            nc.vector.tensor_tensor(
                out=tmpb[:, :, :, :],
                in0=_apf(wv, (s_ - 1) * 4, [[s_ * 4, n], [0, 4], [1, 4]]),
                in1=_apf(Uf[5 - d], 16, [[32, n], [4, 4], [1, 4]]),
                op=mybir.AluOpType.add,
            )
            nc.vector.tensor_reduce(
                out=_apf(wv, (s_ // 2 - 1) * 4, [[s_ * 4, n], [1, 4]]),
                in_=tmpb[:, :, :, :],
                axis=mybir.AxisListType.X, op=mybir.AluOpType.max,
            )

        # ---- tags: argmax_j(alpha + beta), ties -> smallest j ----
        t4 = spool.tile([128, CL, K], F32, tag="t4")
        nc.vector.tensor_tensor(out=t4[:, :, :],
                                in0=_apf(sv, K, [[1, CL * K]]),
                                in1=_apf(wv, 0, [[1, CL * K]]),
                                op=mybir.AluOpType.add)
        rm = spool.tile([128, CL], F32, tag="rm")
        nc.vector.tensor_reduce(out=rm[:, :], in_=t4[:, :, :],
                                axis=mybir.AxisListType.X, op=mybir.AluOpType.max)
        eq = spool.tile([128, CL, K], F32, tag="eq")
        nc.vector.tensor_tensor(out=eq[:, :, :], in0=t4[:, :, :],
                                in1=_apf(rm, 0, [[1, CL], [0, K]]),
                                op=mybir.AluOpType.is_lt)
        t5 = spool.tile([128, CL, K], F32, tag="t5")
        nc.vector.scalar_tensor_tensor(
            out=t5[:, :, :], in0=eq[:, :, :], scalar=BIG,
            in1=_apf(iota_sb, 0, [[0, CL], [1, K]]),
            op0=mybir.AluOpType.mult, op1=mybir.AluOpType.add)
        amn = spool.tile([128, CL], F32, tag="amn")
        nc.vector.tensor_reduce(out=amn[:, :], in_=t5[:, :, :],
                                axis=mybir.AxisListType.X, op=mybir.AluOpType.min)
        tagi = spool.tile([128, CL], I32, tag="tagi")
        nc.vector.tensor_copy(out=tagi[:, :], in_=amn[:, :])
        # pi = b*16 + c: out addr = p*64 + u (fully contiguous)
        nc.sync.dma_start(
            out=_ap(out_tags, 0, [[CL, 128], [1, CL]]),
            in_=tagi[:, :],
        )

    if split_waits:
        _split_multi_waits(nc)
    return nc


def prep_core_inputs(core, sentence_features, conv_w, conv_b, lin_w, lin_b,
                     crf_start, crf_end, crf_trans):
    sf = np.asarray(sentence_features, np.float32)
    conv_w = np.asarray(conv_w, np.float32)
    conv_b = np.asarray(conv_b, np.float32)
    lin_w = np.asarray(lin_w, np.float32)
    lin_b = np.asarray(lin_b, np.float32)
    crf_start = np.asarray(crf_start, np.float32)
    crf_end = np.asarray(crf_end, np.float32)
    crf_trans = np.asarray(crf_trans, np.float32)

    xsh = sf[core * BPC:(core + 1) * BPC]  # [8, T, H]
    xpad = np.zeros((BPC, H, T + 2), np.float32)
    xpad[:, :, 1:T + 1] = xsh.transpose(0, 2, 1)
    xp = np.ascontiguousarray(xpad.reshape(BPC, 2, 128, T + 2)).astype(np.float16)

    wt = conv_w.transpose(1, 0, 2)  # [hin, hout, k]
    wcv = np.empty((12, 128, 128), np.float32)
    for ho in range(2):
        for k in range(3):
            for hi in range(2):
                g = ho * 6 + k * 2 + hi
                wcv[g] = wt[hi * 128:(hi + 1) * 128, ho * 128:(ho + 1) * 128, k]

    transp = crf_trans + lin_b[None, :]
    trans2 = np.empty((4, 4, 4), np.float32)
    for i in range(4):
        for x in range(4):
            for y in range(4):
                trans2[i, x, y] = transp[i, y] + transp[y, x]
    e_mat = np.full((4, 4), NEG, np.float32)
    np.fill_diagonal(e_mat, 0.0)
    # partition map pi = b*16 + c: c = p % 16
    cvec = np.arange(128) % 16
    emask = np.zeros((128, 16), np.float32)
    emask[np.arange(128), cvec] = 1.0
    wmask = np.zeros((128, 16), np.float32)
    wmask[np.arange(128), 15 - cvec] = 1.0
    e0fix = np.zeros((128, 16), np.float32)
    e0fix[cvec == 0] = (e_mat - crf_trans).reshape(16)

    return {
        "xp": xp,
        "wcv": wcv.astype(np.float16),
        "lt": np.ascontiguousarray(lin_w.T.reshape(2, 128, K)).astype(np.float16),
        "cb": conv_b.reshape(2, 128).copy(),
        "trans_r": np.tile(transp.reshape(1, 16), (128, 1)).copy(),
        "trans2_r": np.tile(trans2.reshape(1, 64), (128, 1)).copy(),
        "e0fix_r": e0fix,
        "iota_r": np.tile(np.arange(4, dtype=np.float32), (128, 1)).copy(),
        "emask_r": emask,
        "wmask_r": wmask,
        "start_r": np.tile(crf_start[None, :], (128, 1)).astype(np.float32),
        "end_r": np.tile(crf_end[None, :], (128, 1)).astype(np.float32),
    }


_NC_CACHE = None


def kernel(sentence_features, conv_w, conv_b, lin_w, lin_b, crf_start,
           crf_end, crf_trans):
    global _NC_CACHE
    if _NC_CACHE is None:
        _NC_CACHE = build_program()
    nc = _NC_CACHE
    in_maps = [
        prep_core_inputs(c, sentence_features, conv_w, conv_b, lin_w, lin_b,
                         crf_start, crf_end, crf_trans)
        for c in range(NCORES)
    ]
    res = bass_utils.run_bass_kernel_spmd(nc, in_maps, core_ids=list(range(NCORES)))
    kernel.last_results = res
    out = np.concatenate([res.results[c]["out_tags"] for c in range(NCORES)], axis=0)
    return out.astype(np.int32)
